# revision 8
# baseline (speedup 1.0000x reference)
"""Mixture-of-logistics NLL loss (reduction=mean) on 8 Trainium2 NeuronCores.

Math (per row, K=16 mixture components):
    log_prob = ln(sum_k e^{w_k} pdf_k) - ln(sum_k e^{w_k})
    pdf_k = logistic_pdf(t; loc_k, s_k) = rp_k * sech^2(z_k/2) / 4,
            z_k = (t - loc_k) * rp_k,  rp = 1/s
    sech^2(z/2) = 1 - tanh^2(z/2)
The 1/4 factor is pulled out of the per-row sum and folded into the host
combine as a single -ln(4).

Design (v6; evolved via hardware traces of v1..v4):
Measured engine rates (c=256 chunk, [128,256,16] tiles):
  DVE tensor_tensor bf16 2x        2.29us   (broadcast AP or mixed dtype: 1x)
  DVE tensor_scalar bf16 4x        1.22us
  DVE scalar_tensor_tensor         3.95us   (1x only -> NOT used)
  DVE tensor_reduce                1x       (2x slower than tree16 -> tree16)
  DVE reciprocal_approx_fast       3.94us
  ACT any activation               3.33us   (1 elem/cycle, dtype-blind)
  ACT table-set switch             2.7us
Both ACT and DVE must stay under the ~125us DMA floor (51.4MB f32 per
core at ~410GB/s), so the work is split:
 - ACT (per chunk): tbc = Copy(t broadcast over K)  [lifts the t-loc
   broadcast subtract off DVE, where stride-0 APs run at 1x],
   Exp(w), Tanh(z/2), Square -- all in the one `exp_and_others` set.
 - rp = 1/s alternates per chunk (engine balancing, 'A'/'D' paths):
     A: u = Ln(s); rp = Exp(-u)      [ACT; ln forces the set-6 table, so
        A-chunk Lns/Exps are BATCHED at the head of each group of 4
        chunks -> 2 table loads per group instead of 2 per chunk]
     D: rp = reciprocal_approx_fast  [custom DVE op on raw f32 scale]
 - DVE (per chunk): diff = tbc - loc (2x), z = diff*rp, pw = rp*e^w,
   nsq = 1 - th^2 (tensor_scalar), term = nsq*pw, tree16 row-sums of
   e^w and term.
 - ACT order is pinned with scheduler-only deps to keep the table-set
   batching; w/loc/t stream as f32->bf16 SWDGE cast DMAs on gpsimd
   (GpSimd does only descgen: its tensor ops lock the SBUF port shared
   with DVE); the D-path scale streams raw f32 on the HWDGE sync queue.
 - Validated end-to-end ~3e-4 rel error vs the fp jax reference.

Sharding: pure data parallel over rows (batch*seq) across 8 cores; each core
returns per-partition partial sums [128, 2] = (sum ln(num), sum ln(den));
host combines (mean - ln 4).
"""

import numpy as np

import concourse.bacc as bacc
import concourse.mybir as mybir
import concourse.tile as tile
from concourse.tile_rust import add_dep_helper
from concourse.bass_utils import run_bass_kernel_spmd

B, T, K = 16, 131072, 16
N = B * T                 # 2097152 rows total
NCORES = 8
NLOC = N // NCORES        # 262144 rows per core
P = 128                   # SBUF partitions

F32 = mybir.dt.float32
BF16 = mybir.dt.bfloat16
AF = mybir.ActivationFunctionType
OP = mybir.AluOpType


def build_kernel(nloc=NLOC, groups=None):
    """Build the per-core Bass module.

    groups: list of groups; each group is a list of (rows, path) with
    path 'A' (ACT ln/exp reciprocal) or 'D' (custom-DVE reciprocal).
    """
    p = P
    r = nloc // p             # rows per partition (2048)
    if groups is None:
        g128 = [(128, "A"), (128, "D"), (128, "A"), (128, "D")]
        g256 = [(256, "A"), (256, "D"), (256, "A"), (256, "D")]
        groups = [g128, g256, g128]
    chunks = [c for g in groups for c, _ in g]
    assert sum(chunks) == r and nloc % p == 0
    cmax = max(chunks)

    nc = bacc.Bacc("TRN2", target_bir_lowering=False, debug=False)
    w_d = nc.dram_tensor("w", [nloc, K], F32, kind="ExternalInput")
    loc_d = nc.dram_tensor("loc", [nloc, K], F32, kind="ExternalInput")
    scale_d = nc.dram_tensor("scale", [nloc, K], F32, kind="ExternalInput")
    t_d = nc.dram_tensor("t", [nloc], F32, kind="ExternalInput")
    out_d = nc.dram_tensor("out", [p, 2], F32, kind="ExternalOutput")

    wv = w_d.ap().rearrange("(p r) k -> p r k", p=p)
    lv = loc_d.ap().rearrange("(p r) k -> p r k", p=p)
    sv = scale_d.ap().rearrange("(p r) k -> p r k", p=p)
    tv = t_d.ap().rearrange("(p r) -> p r", p=p)

    acts = []  # every ACT instruction, in required execution order

    def act(*args, **kwargs):
        ins = nc.scalar.activation(*args, **kwargs)
        acts.append(ins)
        return ins

    with tile.TileContext(nc) as tc:
        with (
            tc.tile_pool(name="persist", bufs=1) as pp,
            tc.tile_pool(name="prp", bufs=5) as prp,     # bf16 scale -> rp
            tc.tile_pool(name="psc32", bufs=3) as psc32,  # f32 scale (D path)
            tc.tile_pool(name="pwld", bufs=3) as pwld,
            tc.tile_pool(name="plc", bufs=3) as plc,
            tc.tile_pool(name="ptb", bufs=3) as ptb,
            tc.tile_pool(name="pt", bufs=2) as pt,
            nc.allow_low_precision("bf16 pipeline validated: ~3e-4 rel"),
        ):
            t_all = pp.tile([p, r], BF16)         # targets (bf16)
            stash_n = pp.tile([p, r], F32)        # per-row numerator sums
            stash_d = pp.tile([p, r], F32)        # per-row denominator sums
            out_sb = pp.tile([p, 2], F32)

            nc.gpsimd.dma_start(out=t_all, in_=tv)   # one cast DMA up front

            def tree16(src, dst_slice, c):
                """Sum src [p, c, 16] bf16 over last axis -> dst_slice [p, c] f32."""
                t1 = pt.tile([p, cmax, 8], BF16, tag="t1", name="t1")[:, :c, :]
                nc.vector.tensor_add(out=t1, in0=src[:, :, 0:8], in1=src[:, :, 8:16])
                t2 = pt.tile([p, cmax, 4], BF16, tag="t2", name="t2")[:, :c, :]
                nc.vector.tensor_add(out=t2, in0=t1[:, :, 0:4], in1=t1[:, :, 4:8])
                t3 = pt.tile([p, cmax, 2], BF16, tag="t3", name="t3")[:, :c, :]
                nc.vector.tensor_add(out=t3, in0=t2[:, :, 0:2], in1=t2[:, :, 2:4])
                nc.vector.tensor_add(out=dst_slice, in0=t3[:, :, 0], in1=t3[:, :, 1])

            from concourse.dve_ops import (
                RECIP_APPROX_FAST_CONSTS,
                RECIPROCAL_APPROX_FAST,
            )
            cns = RECIP_APPROX_FAST_CONSTS

            off = 0
            for g in groups:
                # ---- group head: scale DMAs + reciprocal production ----
                ginfo = []
                o = off
                for c, path in g:
                    sl = slice(o, o + c)
                    o += c
                    rp_t = prp.tile([p, cmax, K], BF16, tag="rp", name="rpt")[:, :c, :]
                    sc32 = None
                    if path == "A":
                        nc.gpsimd.dma_start(out=rp_t, in_=sv[:, sl, :])  # bf16 cast
                    else:
                        sc32 = psc32.tile([p, cmax, K], F32, tag="s32",
                                          name="s32t")[:, :c, :]
                        nc.sync.dma_start(out=sc32, in_=sv[:, sl, :])    # raw f32
                    ginfo.append((sl, c, path, rp_t, sc32))

                # A path: batched Ln then Exp(-u) (both resolved in set 6)
                for sl, c, path, rp_t, sc32 in ginfo:
                    if path == "A":
                        act(out=rp_t, in_=rp_t, func=AF.Ln)
                for sl, c, path, rp_t, sc32 in ginfo:
                    if path == "A":
                        act(out=rp_t, in_=rp_t, func=AF.Exp, scale=-1.0)
                # D path: one custom DVE op, f32 in -> bf16 out
                for sl, c, path, rp_t, sc32 in ginfo:
                    if path == "D":
                        nc.vector._custom_dve(
                            RECIPROCAL_APPROX_FAST, out=rp_t, in0=sc32,
                            s0=cns["s0"], s1=cns["s1"], imm2=cns["imm2"],
                        )

                # ---- chunk bodies ----
                for sl, c, path, rp_t, sc32 in ginfo:
                    w_t = pwld.tile([p, cmax, K], BF16, tag="w", name="wt")[:, :c, :]
                    loc_t = plc.tile([p, cmax, K], BF16, tag="loc", name="loct")[:, :c, :]
                    tbc = ptb.tile([p, cmax, K], BF16, tag="tb", name="tbt")[:, :c, :]
                    nc.gpsimd.dma_start(out=w_t, in_=wv[:, sl, :])
                    nc.gpsimd.dma_start(out=loc_t, in_=lv[:, sl, :])

                    act(out=w_t, in_=w_t, func=AF.Exp)               # e^w
                    tb = t_all[:, sl].unsqueeze(2).broadcast_to([p, c, K])
                    act(out=tbc, in_=tb, func=AF.Copy)               # t bcast

                    nc.vector.tensor_sub(out=loc_t, in0=tbc, in1=loc_t)  # diff
                    nc.vector.tensor_mul(out=loc_t, in0=loc_t, in1=rp_t)  # z

                    act(out=loc_t, in_=loc_t, func=AF.Tanh, scale=0.5)   # th
                    act(out=loc_t, in_=loc_t, func=AF.Square)            # th^2

                    tree16(w_t, stash_d[:, sl], c)                   # sum e^w
                    nc.vector.tensor_mul(out=rp_t, in0=rp_t, in1=w_t)    # pw
                    nc.vector.tensor_scalar(
                        out=loc_t, in0=loc_t, scalar1=-1.0, scalar2=1.0,
                        op0=OP.mult, op1=OP.add,
                    )                                                # 1 - th^2
                    nc.vector.tensor_mul(out=loc_t, in0=loc_t, in1=rp_t)  # term
                    tree16(loc_t, stash_n[:, sl], c)
                off = o

            # ---- per-row logs + per-partition accumulation ----
            act(out=stash_n, in_=stash_n, func=AF.Ln, accum_out=out_sb[:, 0:1])
            act(out=stash_d, in_=stash_d, func=AF.Ln, accum_out=out_sb[:, 1:2])
            nc.gpsimd.dma_start(out=out_d.ap(), in_=out_sb)

            # Pin ACT execution order (same engine -> scheduler-only edges)
            for prev, nxt in zip(acts, acts[1:]):
                add_dep_helper(nxt.ins, prev.ins, False, "act-table-order")

    nc.compile()
    return nc


def _combine(outs, n_rows):
    total = 0.0
    for o in outs:
        total += float(o[:, 0].sum(dtype=np.float64))
        total -= float(o[:, 1].sum(dtype=np.float64))
    return np.float32(total / n_rows - np.log(4.0))


def make_in_maps(weight, loc, scale, targets):
    w = np.ascontiguousarray(weight.reshape(N, K), dtype=np.float32)
    l = np.ascontiguousarray(loc.reshape(N, K), dtype=np.float32)
    s = np.ascontiguousarray(scale.reshape(N, K), dtype=np.float32)
    t = np.ascontiguousarray(targets.reshape(N), dtype=np.float32)
    in_maps = []
    for ci in range(NCORES):
        rs = slice(ci * NLOC, (ci + 1) * NLOC)
        in_maps.append({
            "w": np.ascontiguousarray(w[rs]),
            "loc": np.ascontiguousarray(l[rs]),
            "scale": np.ascontiguousarray(s[rs]),
            "t": np.ascontiguousarray(t[rs]),
        })
    return in_maps


def run(in_maps, **kwargs):
    nc = build_kernel()
    return run_bass_kernel_spmd(nc, in_maps, core_ids=list(range(NCORES)), **kwargs)


def kernel(weight, loc, scale, targets):
    in_maps = make_in_maps(weight, loc, scale, targets)
    last = None
    for _ in range(3):  # rare transient NRT device errors: retry
        try:
            res = run(in_maps)
            return _combine([r["out"] for r in res.results], N)
        except Exception as e:  # noqa: BLE001
            last = e
    raise last


if __name__ == "__main__":
    nc = build_kernel()
    print("kernel built OK")


# revision 9
# speedup vs baseline: 1.1814x; 1.1814x over previous
"""Mixture-of-logistics NLL loss (reduction=mean) on 8 Trainium2 NeuronCores.

Math (per row, K=16 mixture components):
    log_prob = ln(sum_k e^{w_k} pdf_k) - ln(sum_k e^{w_k})
    pdf_k = logistic_pdf(t; loc_k, s_k) = rp_k * sech^2(z_k/2) / 4,
            z_k = (t - loc_k) * rp_k,  rp = 1/s
    sech^2(z/2) = 1 - tanh^2(z/2)
The 1/4 factor is pulled out of the per-row sum and folded into the host
combine as a single -ln(4).

Design (v7; evolved via hardware traces of v1..v6):
Measured engine rates (c=256 chunk, [128,256,16] bf16 tiles):
  DVE tensor_tensor 2x  2.29us | broadcast/mixed-dtype 1x  4.42us
  DVE tensor_scalar 4x  1.22us | scalar_tensor_tensor 1x (unused)
  DVE tensor_reduce 1x (tree16 is 2x faster -> tree16)
  DVE reciprocal_approx_fast (custom op, f32 in bf16 out)  ~4.3us
  ACT any activation ~3.7us (1 elem/cycle + ~290ns overhead, dtype-blind)
  ACT table-set switch ~2.7us
Work split (both engines end up ~equally loaded, just above the ~125us
DMA floor of 51.4MB f32/core at ~410GB/s):
 - ACT: tbc = Copy(t broadcast over K) [keeps the t-loc subtract at 2x
   on DVE], Exp(w), Tanh(z/2), Square -- all in the `exp_and_others`
   table set.  1/s runs on ACT (Ln + Exp(-u), set 6) for the 'A' chunks
   only (engine balancing), batched at the head of each 4-chunk group so
   the set6<->set0 rotation costs 2 loads per group.
 - DVE: 'D'-chunk 1/s via one custom reciprocal_approx_fast op (f32
   scale streamed on the HWDGE sync queue), diff = tbc - loc, z =
   diff*rp, pw = rp*e^w, nsq = 1-th^2 (tensor_scalar), term = nsq*pw,
   tree16 row-sums of e^w and term.
 - Anti-ping-pong: within each group the ACT chain runs one chunk ahead
   of the Tanh/Square pair ([ExpW_i, Copy_i, Tanh_{i-1}, Sq_{i-1}]), and
   the DVE chain is split PRE_i = [recip, diff, z, treeW, pw] / POST_i =
   [nsq, term, treeS] with the same one-chunk trail, so neither engine
   stalls on the other's current chunk.  ACT order is pinned with
   scheduler-only deps (table batching + lookahead).
 - w/loc/t stream as f32->bf16 SWDGE cast DMAs on gpsimd (GpSimd does
   only descgen: its tensor ops lock the SBUF port shared with DVE).
 - Validated end-to-end ~3e-4 rel error vs the fp jax reference.

Sharding: pure data parallel over rows (batch*seq) across 8 cores; each core
returns per-partition partial sums [128, 2] = (sum ln(num), sum ln(den));
host combines (mean - ln 4).
"""

import numpy as np

import concourse.bacc as bacc
import concourse.mybir as mybir
import concourse.tile as tile
from concourse.tile_rust import add_dep_helper
from concourse.bass_utils import run_bass_kernel_spmd

B, T, K = 16, 131072, 16
N = B * T                 # 2097152 rows total
NCORES = 8
NLOC = N // NCORES        # 262144 rows per core
P = 128                   # SBUF partitions

F32 = mybir.dt.float32
BF16 = mybir.dt.bfloat16
AF = mybir.ActivationFunctionType
OP = mybir.AluOpType


def build_kernel(nloc=NLOC, groups=None):
    """Build the per-core Bass module.

    groups: list of groups; each group is a list of (rows, path) with
    path 'A' (ACT ln/exp reciprocal) or 'D' (custom-DVE reciprocal).
    """
    p = P
    r = nloc // p             # rows per partition (2048)
    if groups is None:
        groups = [
            [(128, "A"), (128, "D"), (128, "D"), (128, "A")],
            [(256, "D"), (256, "D"), (256, "D"), (256, "D")],
            [(128, "A"), (128, "D"), (128, "D"), (128, "A")],
        ]
    chunks = [c for g in groups for c, _ in g]
    assert sum(chunks) == r and nloc % p == 0
    cmax = max(chunks)

    nc = bacc.Bacc("TRN2", target_bir_lowering=False, debug=False)
    w_d = nc.dram_tensor("w", [nloc, K], F32, kind="ExternalInput")
    loc_d = nc.dram_tensor("loc", [nloc, K], F32, kind="ExternalInput")
    scale_d = nc.dram_tensor("scale", [nloc, K], F32, kind="ExternalInput")
    t_d = nc.dram_tensor("t", [nloc], F32, kind="ExternalInput")
    out_d = nc.dram_tensor("out", [p, 2], F32, kind="ExternalOutput")

    wv = w_d.ap().rearrange("(p r) k -> p r k", p=p)
    lv = loc_d.ap().rearrange("(p r) k -> p r k", p=p)
    sv = scale_d.ap().rearrange("(p r) k -> p r k", p=p)
    tv = t_d.ap().rearrange("(p r) -> p r", p=p)

    acts = []  # every ACT instruction, in required execution order

    def act(*args, **kwargs):
        ins = nc.scalar.activation(*args, **kwargs)
        acts.append(ins)
        return ins

    with tile.TileContext(nc) as tc:
        with (
            tc.tile_pool(name="persist", bufs=1) as pp,
            tc.tile_pool(name="prp", bufs=4) as prp,     # bf16 scale -> rp
            tc.tile_pool(name="psc32", bufs=2) as psc32,  # f32 scale (D path)
            tc.tile_pool(name="pwld", bufs=4) as pwld,
            tc.tile_pool(name="plc", bufs=4) as plc,
            tc.tile_pool(name="ptb", bufs=3) as ptb,
            tc.tile_pool(name="pt", bufs=2) as pt,
            nc.allow_low_precision("bf16 pipeline validated: ~3e-4 rel"),
        ):
            t_all = pp.tile([p, r], BF16)         # targets (bf16)
            stash_n = pp.tile([p, r], F32)        # per-row numerator sums
            stash_d = pp.tile([p, r], F32)        # per-row denominator sums
            out_sb = pp.tile([p, 2], F32)

            nc.gpsimd.dma_start(out=t_all, in_=tv)   # one cast DMA up front

            def tree16(src, dst_slice, c):
                """Sum src [p, c, 16] bf16 over last axis -> dst_slice [p, c] f32."""
                t1 = pt.tile([p, cmax, 8], BF16, tag="t1", name="t1")[:, :c, :]
                nc.vector.tensor_add(out=t1, in0=src[:, :, 0:8], in1=src[:, :, 8:16])
                t2 = pt.tile([p, cmax, 4], BF16, tag="t2", name="t2")[:, :c, :]
                nc.vector.tensor_add(out=t2, in0=t1[:, :, 0:4], in1=t1[:, :, 4:8])
                t3 = pt.tile([p, cmax, 2], BF16, tag="t3", name="t3")[:, :c, :]
                nc.vector.tensor_add(out=t3, in0=t2[:, :, 0:2], in1=t2[:, :, 2:4])
                nc.vector.tensor_add(out=dst_slice, in0=t3[:, :, 0], in1=t3[:, :, 1])

            from concourse.dve_ops import (
                RECIP_APPROX_FAST_CONSTS,
                RECIPROCAL_APPROX_FAST,
            )
            cns = RECIP_APPROX_FAST_CONSTS

            def emit_pre(sl, c, path, rp_t, sc32):
                """ACT lookahead pair + DVE pre-tanh chain for one chunk."""
                w_t = pwld.tile([p, cmax, K], BF16, tag="w", name="wt")[:, :c, :]
                loc_t = plc.tile([p, cmax, K], BF16, tag="loc", name="loct")[:, :c, :]
                tbc = ptb.tile([p, cmax, K], BF16, tag="tb", name="tbt")[:, :c, :]
                nc.gpsimd.dma_start(out=w_t, in_=wv[:, sl, :])
                nc.gpsimd.dma_start(out=loc_t, in_=lv[:, sl, :])

                act(out=w_t, in_=w_t, func=AF.Exp)               # e^w
                tb = t_all[:, sl].unsqueeze(2).broadcast_to([p, c, K])
                act(out=tbc, in_=tb, func=AF.Copy)               # t bcast

                if path == "D":
                    nc.vector._custom_dve(
                        RECIPROCAL_APPROX_FAST, out=rp_t, in0=sc32,
                        s0=cns["s0"], s1=cns["s1"], imm2=cns["imm2"],
                    )
                nc.vector.tensor_sub(out=loc_t, in0=tbc, in1=loc_t)   # diff
                nc.vector.tensor_mul(out=loc_t, in0=loc_t, in1=rp_t)  # z
                tree16(w_t, stash_d[:, sl], c)                    # sum e^w
                nc.vector.tensor_mul(out=rp_t, in0=rp_t, in1=w_t)     # pw
                return loc_t, rp_t

            def emit_tanh(pend):
                sl, c, loc_t, pw_t = pend
                act(out=loc_t, in_=loc_t, func=AF.Tanh, scale=0.5)    # th
                act(out=loc_t, in_=loc_t, func=AF.Square)             # th^2

            def emit_post(pend):
                sl, c, loc_t, pw_t = pend
                nc.vector.tensor_scalar(
                    out=loc_t, in0=loc_t, scalar1=-1.0, scalar2=1.0,
                    op0=OP.mult, op1=OP.add,
                )                                                 # 1 - th^2
                nc.vector.tensor_mul(out=loc_t, in0=loc_t, in1=pw_t)  # term
                tree16(loc_t, stash_n[:, sl], c)

            off = 0
            for g in groups:
                # ---- group head: scale DMAs + A-path reciprocal batch ----
                ginfo = []
                o = off
                for c, path in g:
                    sl = slice(o, o + c)
                    o += c
                    rp_t = prp.tile([p, cmax, K], BF16, tag="rp", name="rpt")[:, :c, :]
                    sc32 = None
                    if path == "A":
                        nc.gpsimd.dma_start(out=rp_t, in_=sv[:, sl, :])  # bf16 cast
                    else:
                        sc32 = psc32.tile([p, cmax, K], F32, tag="s32",
                                          name="s32t")[:, :c, :]
                        nc.sync.dma_start(out=sc32, in_=sv[:, sl, :])    # raw f32
                    ginfo.append((sl, c, path, rp_t, sc32))

                for sl, c, path, rp_t, sc32 in ginfo:
                    if path == "A":
                        act(out=rp_t, in_=rp_t, func=AF.Ln)
                for sl, c, path, rp_t, sc32 in ginfo:
                    if path == "A":
                        act(out=rp_t, in_=rp_t, func=AF.Exp, scale=-1.0)

                # ---- chunk bodies, software-pipelined one deep ----
                pend = None
                for sl, c, path, rp_t, sc32 in ginfo:
                    loc_t, pw_t = emit_pre(sl, c, path, rp_t, sc32)
                    if pend is not None:
                        emit_tanh(pend)
                        emit_post(pend)
                    pend = (sl, c, loc_t, pw_t)
                emit_tanh(pend)     # flush before next group's set-6 batch
                emit_post(pend)
                off = o

            # ---- per-row logs + per-partition accumulation ----
            act(out=stash_n, in_=stash_n, func=AF.Ln, accum_out=out_sb[:, 0:1])
            act(out=stash_d, in_=stash_d, func=AF.Ln, accum_out=out_sb[:, 1:2])
            nc.gpsimd.dma_start(out=out_d.ap(), in_=out_sb)

            # Pin ACT execution order (same engine -> scheduler-only edges)
            for prev, nxt in zip(acts, acts[1:]):
                add_dep_helper(nxt.ins, prev.ins, False, "act-table-order")

    nc.compile()
    return nc


def _combine(outs, n_rows):
    total = 0.0
    for o in outs:
        total += float(o[:, 0].sum(dtype=np.float64))
        total -= float(o[:, 1].sum(dtype=np.float64))
    return np.float32(total / n_rows - np.log(4.0))


def make_in_maps(weight, loc, scale, targets):
    w = np.ascontiguousarray(weight.reshape(N, K), dtype=np.float32)
    l = np.ascontiguousarray(loc.reshape(N, K), dtype=np.float32)
    s = np.ascontiguousarray(scale.reshape(N, K), dtype=np.float32)
    t = np.ascontiguousarray(targets.reshape(N), dtype=np.float32)
    in_maps = []
    for ci in range(NCORES):
        rs = slice(ci * NLOC, (ci + 1) * NLOC)
        in_maps.append({
            "w": np.ascontiguousarray(w[rs]),
            "loc": np.ascontiguousarray(l[rs]),
            "scale": np.ascontiguousarray(s[rs]),
            "t": np.ascontiguousarray(t[rs]),
        })
    return in_maps


def run(in_maps, **kwargs):
    nc = build_kernel()
    return run_bass_kernel_spmd(nc, in_maps, core_ids=list(range(NCORES)), **kwargs)


def kernel(weight, loc, scale, targets):
    in_maps = make_in_maps(weight, loc, scale, targets)
    last = None
    for _ in range(3):  # rare transient NRT device errors: retry
        try:
            res = run(in_maps)
            return _combine([r["out"] for r in res.results], N)
        except Exception as e:  # noqa: BLE001
            last = e
    raise last


if __name__ == "__main__":
    nc = build_kernel()
    print("kernel built OK")
